# revision 25
# baseline (speedup 1.0000x reference)
"""Cumulative-probability head on 8 Trainium2 NeuronCores.

out[b, j] = sum_{i<=j} relu(x @ W_h^T + b_h)[b, i] + (x @ W_base^T + b_base)[b]

Data-parallel: x is sharded along batch (1024 rows per core); the small
weights are replicated. The host passes x pre-transposed per shard
([D, 1024], contiguous rows) so the contraction dim lands on SBUF
partitions with no on-device transposes. Per core:

  1. haz = xT.T @ WT_aug in float32r (FP22 multiplies, fp32 PSUM accum).
     WT_aug is [2049, 516]: hazard cols 0..511, base col 512, 3 zero pad
     cols; row 2048 is the bias row, added post-accumulation on DVE
     via a partition-broadcast read. The 516 output cols split into two
     even N=258 PSUM banks (fp32r requires an even moving dim).
  2. Each 128-row batch tile accumulates the full K=2048 contraction
     in a pair of PSUM banks; four tiles are in flight (8 banks), so
     the PE has work while input chunks stream in.
  3. Input DMAs are spread over three queue rings (Sync HWDGE, Scalar
     HWDGE, GPSIMD SWDGE) in k-order — one ring caps at ~160 GB/s,
     which would serialize the 12 MB of per-core input.
  4. ReLU on ScalarE (hazard cols only; base col stays unactivated),
     then the inclusive cumsum along T on DVE via tensor_tensor_scan
     with the base hazard as the per-partition initial state.
"""

import numpy as np

import concourse.bass as bass
import concourse.tile as tile
from concourse import bacc, mybir
from concourse.bass_utils import run_bass_kernel_spmd

B, D, T = 8192, 2048, 512
NCORES = 8
BLOC = B // NCORES            # 1024 rows per core
NB = BLOC // 128              # 8 batch tiles per core
NK = D // 128                 # 16 contraction chunks
TP = 516                      # padded output width (512 hazard + base + 3 junk)
NA = 258                      # output cols 0..257 in PSUM bank A
NBK = TP - NA                 # cols 258..515 in bank B (col 512 = base)
BOFF = T - NA                 # offset of the base col inside bank B (254)

F32 = mybir.dt.float32
F32R = mybir.dt.float32r


def _build_program():
    nc = bacc.Bacc("TRN2", target_bir_lowering=False, debug=False)

    xt_d = nc.dram_tensor("xt", [D, BLOC], F32R, kind="ExternalInput")
    wt_d = nc.dram_tensor("wt", [D + 1, TP], F32R, kind="ExternalInput")
    out_d = nc.dram_tensor("out", [BLOC, T], F32, kind="ExternalOutput")

    with tile.TileContext(nc) as tc:
        with (
            tc.tile_pool(name="consts", bufs=1) as consts,
            tc.tile_pool(name="wt", bufs=1) as wtp,
            tc.tile_pool(name="xt", bufs=1) as xtp,
            tc.tile_pool(name="haz", bufs=4) as hazp,
            tc.tile_pool(name="part", bufs=1) as partp,
            tc.tile_pool(name="outp", bufs=4) as outp,
            tc.tile_pool(name="ps_mm", bufs=4, space="PSUM") as ps_mm,
        ):
            zeros = consts.tile([128, T], F32)
            nc.vector.memset(zeros, 0.0)

            # Input loads, k-ordered, spread over three DMA rings. The two
            # HWDGE rings (Sync, Scalar) are faster than the GPSIMD SWDGE
            # ring, so the first chunks — which gate the PE pipeline ramp —
            # go to the HWDGE rings, and the slow ring only carries late
            # chunks. Weights (half the size of an x chunk) ride opposite
            # rings from their x chunk so the pair lands together.
            XT_RING = [0, 1, 0, 1, 2, 0, 1, 2, 0, 1, 2, 0, 1, 2, 0, 1]
            WT_RING = [1, 0, 1, 0, 0, 1, 2, 0, 1, 2, 0, 1, 2, 0, 1, 2]
            rings = [nc.sync, nc.scalar, nc.gpsimd]
            xt_tiles = []
            wt_tiles = []
            for k in range(NK):
                xk = xtp.tile([128, BLOC], F32R, tag=f"xt{k}")
                rings[XT_RING[k]].dma_start(out=xk, in_=xt_d[128 * k : 128 * (k + 1), :])
                xt_tiles.append(xk)
                w = wtp.tile([128, TP], F32R, tag=f"wt{k}")
                rings[WT_RING[k]].dma_start(out=w, in_=wt_d[128 * k : 128 * (k + 1), :])
                wt_tiles.append(w)
            # Bias row replicated across all 128 partitions with a
            # partition-stride-0 DMA read (engines can't read stride-0
            # partition APs, but DMA can).
            wbias_bc = wtp.tile([128, TP], F32, tag="wbias")
            bias_src = wt_d[D : D + 1, :]
            nc.gpsimd.dma_start(
                out=wbias_bc,
                in_=bass.AP(
                    tensor=bias_src.tensor,
                    offset=bias_src.offset,
                    ap=[[0, 128]] + list(bias_src.ap[1:]),
                ).bitcast(F32),
            )

            # K-split into two PSUM generations sharing one tag pair at
            # bufs=4: four accumulators live at any moment, but each only
            # spans half the K range, so banks recycle mid-window and the
            # second half of the batch tiles overlaps the input-DMA window
            # instead of queuing behind it.
            NK2 = NK // 2
            partials = []
            for b in range(NB):
                pA = ps_mm.tile([128, NA], F32, tag="pA")
                pB = ps_mm.tile([128, NBK], F32, tag="pB")
                for k in range(NK2):
                    xt_ap = xt_tiles[k][:, 128 * b : 128 * (b + 1)]
                    w = wt_tiles[k]
                    nc.tensor.matmul(
                        pA[:], xt_ap, w[:, 0:NA],
                        start=(k == 0), stop=(k == NK2 - 1),
                    )
                    nc.tensor.matmul(
                        pB[:], xt_ap, w[:, NA:TP],
                        start=(k == 0), stop=(k == NK2 - 1),
                    )
                part = partp.tile([128, 2, NA], F32, tag=f"part{b}")
                nc.scalar.copy(out=part[:, 0, :], in_=pA[:])
                nc.scalar.copy(out=part[:, 1, :], in_=pB[:])
                partials.append(part)

            for b in range(NB):
                pA = ps_mm.tile([128, NA], F32, tag="pA")
                pB = ps_mm.tile([128, NBK], F32, tag="pB")
                for kk in range(NK2):
                    k = NK2 + kk
                    xt_ap = xt_tiles[k][:, 128 * b : 128 * (b + 1)]
                    w = wt_tiles[k]
                    nc.tensor.matmul(
                        pA[:], xt_ap, w[:, 0:NA],
                        start=(kk == 0), stop=(kk == NK2 - 1),
                    )
                    nc.tensor.matmul(
                        pB[:], xt_ap, w[:, NA:TP],
                        start=(kk == 0), stop=(kk == NK2 - 1),
                    )

                part = partials[b]
                pre = hazp.tile([128, 2, NA], F32, tag="pre")
                nc.vector.tensor_add(pre[:, 0, :], pA[:], part[:, 0, :])
                nc.vector.tensor_add(pre[:, 1, :], pB[:], part[:, 1, :])
                nc.vector.tensor_add(pre[:, 0, :], pre[:, 0, :], wbias_bc[:, 0:NA])
                nc.vector.tensor_add(pre[:, 1, :], pre[:, 1, :], wbias_bc[:, NA:TP])

                haz = hazp.tile([128, T], F32, tag="haz")
                base = hazp.tile([128, 1], F32, tag="base")
                nc.scalar.activation(
                    out=haz[:, 0:NA], in_=pre[:, 0, :],
                    func=mybir.ActivationFunctionType.Relu,
                )
                nc.scalar.activation(
                    out=haz[:, NA:T], in_=pre[:, 1, 0:BOFF],
                    func=mybir.ActivationFunctionType.Relu,
                )
                nc.scalar.copy(out=base, in_=pre[:, 1, BOFF : BOFF + 1])

                cum = outp.tile([128, T], F32)
                nc.vector.tensor_tensor_scan(
                    out=cum,
                    data0=haz,
                    data1=zeros,
                    initial=base,
                    op0=mybir.AluOpType.add,
                    op1=mybir.AluOpType.add,
                )
                nc.scalar.dma_start(out=out_d[128 * b : 128 * (b + 1), :], in_=cum)

    nc.compile()
    return nc


_NC_CACHE = None


def kernel(x, W_hazard, b_hazard, W_base, b_base):
    global _NC_CACHE
    if _NC_CACHE is None:
        _NC_CACHE = _build_program()
    nc = _NC_CACHE

    x = np.asarray(x, dtype=np.float32)
    W_cat = np.concatenate(
        [np.asarray(W_hazard, np.float32), np.asarray(W_base, np.float32)], axis=0
    )  # [513, 2048]
    bias_row = np.concatenate(
        [np.asarray(b_hazard, np.float32), np.asarray(b_base, np.float32)]
    )  # [513]
    wt = np.concatenate([W_cat.T, bias_row[None, :]], axis=0)  # [2049, 513]
    wt = np.ascontiguousarray(
        np.concatenate([wt, np.zeros((D + 1, TP - (T + 1)), np.float32)], axis=1)
    )  # [2049, 516]

    in_maps = [
        {
            "xt": np.ascontiguousarray(x[BLOC * i : BLOC * (i + 1)].T),
            "wt": wt,
        }
        for i in range(NCORES)
    ]
    res = run_bass_kernel_spmd(nc, in_maps, list(range(NCORES)))
    return np.concatenate([res.results[i]["out"] for i in range(NCORES)], axis=0)


# revision 26
# speedup vs baseline: 1.0638x; 1.0638x over previous
"""Cumulative-probability head on 8 Trainium2 NeuronCores.

out[b, j] = sum_{i<=j} relu(x @ W_h^T + b_h)[b, i] + (x @ W_base^T + b_base)[b]

Data-parallel: x is sharded along batch (1024 rows per core); the small
weights are replicated. The host passes x pre-transposed per shard
([D, 1024], contiguous rows) so the contraction dim lands on SBUF
partitions with no on-device transposes. Per core:

  1. haz = xT.T @ WT_aug in float32r (FP22 multiplies, fp32 PSUM accum).
     WT_aug is [2049, 516]: hazard cols 0..511, base col 512, 3 zero pad
     cols; row 2048 is the bias row, added post-accumulation on DVE
     via a partition-broadcast read. The 516 output cols split into two
     even N=258 PSUM banks (fp32r requires an even moving dim).
  2. Each 128-row batch tile accumulates the full K=2048 contraction
     in a pair of PSUM banks; four tiles are in flight (8 banks), so
     the PE has work while input chunks stream in.
  3. Input DMAs are spread over three queue rings (Sync HWDGE, Scalar
     HWDGE, GPSIMD SWDGE) in k-order — one ring caps at ~160 GB/s,
     which would serialize the 12 MB of per-core input.
  4. ReLU on ScalarE (hazard cols only; base col stays unactivated),
     then the inclusive cumsum along T on DVE via tensor_tensor_scan
     with the base hazard as the per-partition initial state.
"""

import numpy as np

import concourse.bass as bass
import concourse.tile as tile
from concourse import bacc, mybir
from concourse.bass_utils import run_bass_kernel_spmd

B, D, T = 8192, 2048, 512
NCORES = 8
BLOC = B // NCORES            # 1024 rows per core
NB = BLOC // 128              # 8 batch tiles per core
NK = D // 128                 # 16 contraction chunks
TP = 516                      # padded output width (512 hazard + base + 3 junk)
NA = 258                      # output cols 0..257 in PSUM bank A
NBK = TP - NA                 # cols 258..515 in bank B (col 512 = base)
BOFF = T - NA                 # offset of the base col inside bank B (254)

F32 = mybir.dt.float32
F32R = mybir.dt.float32r


def _build_program():
    nc = bacc.Bacc("TRN2", target_bir_lowering=False, debug=False)

    xt_d = nc.dram_tensor("xt", [D, BLOC], F32R, kind="ExternalInput")
    wt_d = nc.dram_tensor("wt", [D + 1, TP], F32R, kind="ExternalInput")
    out_d = nc.dram_tensor("out", [BLOC, T], F32, kind="ExternalOutput")

    with tile.TileContext(nc) as tc:
        with (
            tc.tile_pool(name="consts", bufs=1) as consts,
            tc.tile_pool(name="wt", bufs=1) as wtp,
            tc.tile_pool(name="xt", bufs=1) as xtp,
            tc.tile_pool(name="haz", bufs=4) as hazp,
            tc.tile_pool(name="outp", bufs=4) as outp,
            tc.tile_pool(name="ps_mm", bufs=4, space="PSUM") as ps_mm,
        ):
            zeros = consts.tile([128, T], F32)
            nc.vector.memset(zeros, 0.0)

            # Input loads, k-ordered, spread over three DMA rings. The two
            # HWDGE rings (Sync, Scalar) are faster than the GPSIMD SWDGE
            # ring, so the first chunks — which gate the PE pipeline ramp —
            # go to the HWDGE rings, and the slow ring only carries late
            # chunks. Weights (half the size of an x chunk) ride opposite
            # rings from their x chunk so the pair lands together.
            XT_RING = [0, 1, 0, 1, 2, 0, 1, 2, 0, 1, 2, 0, 1, 2, 0, 1]
            WT_RING = [1, 0, 1, 0, 0, 1, 2, 0, 1, 2, 0, 1, 2, 0, 1, 2]
            rings = [nc.sync, nc.scalar, nc.gpsimd]
            xt_tiles = []
            wt_tiles = []
            for k in range(NK):
                xk = xtp.tile([128, BLOC], F32R, tag=f"xt{k}")
                rings[XT_RING[k]].dma_start(out=xk, in_=xt_d[128 * k : 128 * (k + 1), :])
                xt_tiles.append(xk)
                w = wtp.tile([128, TP], F32R, tag=f"wt{k}")
                rings[WT_RING[k]].dma_start(out=w, in_=wt_d[128 * k : 128 * (k + 1), :])
                wt_tiles.append(w)
            # Bias row replicated across all 128 partitions with a
            # partition-stride-0 DMA read (engines can't read stride-0
            # partition APs, but DMA can).
            wbias_bc = wtp.tile([128, TP], F32, tag="wbias")
            bias_src = wt_d[D : D + 1, :]
            nc.gpsimd.dma_start(
                out=wbias_bc,
                in_=bass.AP(
                    tensor=bias_src.tensor,
                    offset=bias_src.offset,
                    ap=[[0, 128]] + list(bias_src.ap[1:]),
                ).bitcast(F32),
            )

            # Full-K accumulation per 128-row batch tile; bufs=4 on each
            # PSUM tag -> 4 b-tiles in flight across all 8 banks.
            for b in range(NB):
                pA = ps_mm.tile([128, NA], F32, tag="pA")
                pB = ps_mm.tile([128, NBK], F32, tag="pB")
                for k in range(NK):
                    xt_ap = xt_tiles[k][:, 128 * b : 128 * (b + 1)]
                    w = wt_tiles[k]
                    nc.tensor.matmul(
                        pA[:], xt_ap, w[:, 0:NA],
                        start=(k == 0), stop=(k == NK - 1),
                    )
                    nc.tensor.matmul(
                        pB[:], xt_ap, w[:, NA:TP],
                        start=(k == 0), stop=(k == NK - 1),
                    )

                # Bias row added on DVE via a partition-broadcast read —
                # keeps the K=1 ones-row matmuls (and their LDWEIGHTS)
                # off the PE stream.
                pre = hazp.tile([128, 2, NA], F32, tag="pre")
                nc.vector.tensor_add(pre[:, 0, :], pA[:], wbias_bc[:, 0:NA])
                nc.vector.tensor_add(pre[:, 1, :], pB[:], wbias_bc[:, NA:TP])

                haz = hazp.tile([128, T], F32, tag="haz")
                base = hazp.tile([128, 1], F32, tag="base")
                nc.scalar.activation(
                    out=haz[:, 0:NA], in_=pre[:, 0, :],
                    func=mybir.ActivationFunctionType.Relu,
                )
                nc.scalar.activation(
                    out=haz[:, NA:T], in_=pre[:, 1, 0:BOFF],
                    func=mybir.ActivationFunctionType.Relu,
                )
                nc.scalar.copy(out=base, in_=pre[:, 1, BOFF : BOFF + 1])

                cum = outp.tile([128, T], F32)
                nc.vector.tensor_tensor_scan(
                    out=cum,
                    data0=haz,
                    data1=zeros,
                    initial=base,
                    op0=mybir.AluOpType.add,
                    op1=mybir.AluOpType.add,
                )
                nc.scalar.dma_start(out=out_d[128 * b : 128 * (b + 1), :], in_=cum)

    nc.compile()
    return nc


_NC_CACHE = None


def kernel(x, W_hazard, b_hazard, W_base, b_base):
    global _NC_CACHE
    if _NC_CACHE is None:
        _NC_CACHE = _build_program()
    nc = _NC_CACHE

    x = np.asarray(x, dtype=np.float32)
    W_cat = np.concatenate(
        [np.asarray(W_hazard, np.float32), np.asarray(W_base, np.float32)], axis=0
    )  # [513, 2048]
    bias_row = np.concatenate(
        [np.asarray(b_hazard, np.float32), np.asarray(b_base, np.float32)]
    )  # [513]
    wt = np.concatenate([W_cat.T, bias_row[None, :]], axis=0)  # [2049, 513]
    wt = np.ascontiguousarray(
        np.concatenate([wt, np.zeros((D + 1, TP - (T + 1)), np.float32)], axis=1)
    )  # [2049, 516]

    in_maps = [
        {
            "xt": np.ascontiguousarray(x[BLOC * i : BLOC * (i + 1)].T),
            "wt": wt,
        }
        for i in range(NCORES)
    ]
    res = run_bass_kernel_spmd(nc, in_maps, list(range(NCORES)))
    return np.concatenate([res.results[i]["out"] for i in range(NCORES)], axis=0)


# revision 28
# speedup vs baseline: 1.0651x; 1.0012x over previous
"""Cumulative-probability head on 8 Trainium2 NeuronCores.

out[b, j] = sum_{i<=j} relu(x @ W_h^T + b_h)[b, i] + (x @ W_base^T + b_base)[b]

Data-parallel: x is sharded along batch (1024 rows per core); the small
weights are replicated. The host passes x pre-transposed per shard
([D, 1024], contiguous rows) so the contraction dim lands on SBUF
partitions with no on-device transposes. Per core:

  1. haz = xT.T @ WT_aug in float32r (FP22 multiplies, fp32 PSUM accum).
     WT_aug is [2049, 516]: hazard cols 0..511, base col 512, 3 zero pad
     cols; row 2048 is the bias row, added post-accumulation on DVE
     via a partition-broadcast read. The 516 output cols split into two
     even N=258 PSUM banks (fp32r requires an even moving dim).
  2. Each 128-row batch tile accumulates the full K=2048 contraction
     in a pair of PSUM banks; four tiles are in flight (8 banks), so
     the PE has work while input chunks stream in.
  3. Input DMAs are spread over three queue rings (Sync HWDGE, Scalar
     HWDGE, GPSIMD SWDGE) in k-order — one ring caps at ~160 GB/s,
     which would serialize the 12 MB of per-core input.
  4. ReLU on ScalarE (hazard cols only; base col stays unactivated),
     then the inclusive cumsum along T on DVE via tensor_tensor_scan
     with the base hazard as the per-partition initial state.
"""

import numpy as np

import concourse.bass as bass
import concourse.tile as tile
from concourse import bacc, mybir
from concourse.bass_utils import run_bass_kernel_spmd

B, D, T = 8192, 2048, 512
NCORES = 8
BLOC = B // NCORES            # 1024 rows per core
NB = BLOC // 128              # 8 batch tiles per core
NK = D // 128                 # 16 contraction chunks
TP = 516                      # padded output width (512 hazard + base + 3 junk)
NA = 258                      # output cols 0..257 in PSUM bank A
NBK = TP - NA                 # cols 258..515 in bank B (col 512 = base)
BOFF = T - NA                 # offset of the base col inside bank B (254)

F32 = mybir.dt.float32
F32R = mybir.dt.float32r


def _build_program():
    nc = bacc.Bacc("TRN2", target_bir_lowering=False, debug=False)

    xt_d = nc.dram_tensor("xt", [D, BLOC], F32R, kind="ExternalInput")
    wt_d = nc.dram_tensor("wt", [D + 1, TP], F32R, kind="ExternalInput")
    out_d = nc.dram_tensor("out", [BLOC, T], F32, kind="ExternalOutput")

    with tile.TileContext(nc) as tc:
        with (
            tc.tile_pool(name="consts", bufs=1) as consts,
            tc.tile_pool(name="wt", bufs=1) as wtp,
            tc.tile_pool(name="xt", bufs=1) as xtp,
            tc.tile_pool(name="haz", bufs=4) as hazp,
            tc.tile_pool(name="outp", bufs=4) as outp,
            tc.tile_pool(name="ps_mm", bufs=4, space="PSUM") as ps_mm,
        ):
            zeros = consts.tile([128, T], F32)
            nc.vector.memset(zeros, 0.0)

            # Input loads, k-ordered, spread over three DMA rings. The two
            # HWDGE rings (Sync, Scalar) are faster than the GPSIMD SWDGE
            # ring, so the first chunks — which gate the PE pipeline ramp —
            # go to the HWDGE rings, and the slow ring only carries late
            # chunks. Weights (half the size of an x chunk) ride opposite
            # rings from their x chunk so the pair lands together.
            XT_RING = [0, 1, 0, 1, 2, 0, 1, 2, 0, 1, 2, 0, 1, 2, 0, 1]
            WT_RING = [1, 0, 1, 0, 0, 1, 2, 0, 1, 2, 0, 1, 2, 0, 1, 2]
            rings = [nc.sync, nc.scalar, nc.gpsimd]
            xt_tiles = []
            wt_tiles = []
            for k in range(NK):
                xk = xtp.tile([128, BLOC], F32R, tag=f"xt{k}")
                rings[XT_RING[k]].dma_start(out=xk, in_=xt_d[128 * k : 128 * (k + 1), :])
                xt_tiles.append(xk)
                w = wtp.tile([128, TP], F32R, tag=f"wt{k}")
                rings[WT_RING[k]].dma_start(out=w, in_=wt_d[128 * k : 128 * (k + 1), :])
                wt_tiles.append(w)
            # Bias row replicated across all 128 partitions with a
            # partition-stride-0 DMA read (engines can't read stride-0
            # partition APs, but DMA can).
            wbias_bc = wtp.tile([128, TP], F32, tag="wbias")
            bias_src = wt_d[D : D + 1, :]
            nc.gpsimd.dma_start(
                out=wbias_bc,
                in_=bass.AP(
                    tensor=bias_src.tensor,
                    offset=bias_src.offset,
                    ap=[[0, 128]] + list(bias_src.ap[1:]),
                ).bitcast(F32),
            )

            # Full-K accumulation per 128-row batch tile; bufs=4 on each
            # PSUM tag -> 4 b-tiles in flight across all 8 banks.
            for b in range(NB):
                pA = ps_mm.tile([128, NA], F32, tag="pA")
                pB = ps_mm.tile([128, NBK], F32, tag="pB")
                for k in range(NK):
                    xt_ap = xt_tiles[k][:, 128 * b : 128 * (b + 1)]
                    w = wt_tiles[k]
                    nc.tensor.matmul(
                        pA[:], xt_ap, w[:, 0:NA],
                        start=(k == 0), stop=(k == NK - 1),
                    )
                    nc.tensor.matmul(
                        pB[:], xt_ap, w[:, NA:TP],
                        start=(k == 0), stop=(k == NK - 1),
                    )

                # Bias row added on DVE via a partition-broadcast read —
                # keeps the K=1 ones-row matmuls (and their LDWEIGHTS)
                # off the PE stream.
                pre = hazp.tile([128, 2, NA], F32, tag="pre")
                nc.vector.tensor_add(pre[:, 0, :], pA[:], wbias_bc[:, 0:NA])
                nc.vector.tensor_add(pre[:, 1, :], pB[:], wbias_bc[:, NA:TP])

                haz = hazp.tile([128, T], F32, tag="haz")
                base = hazp.tile([128, 1], F32, tag="base")
                nc.scalar.activation(
                    out=haz[:, 0:NA], in_=pre[:, 0, :],
                    func=mybir.ActivationFunctionType.Relu,
                )
                nc.scalar.activation(
                    out=haz[:, NA:T], in_=pre[:, 1, 0:BOFF],
                    func=mybir.ActivationFunctionType.Relu,
                )
                nc.scalar.copy(out=base, in_=pre[:, 1, BOFF : BOFF + 1])

                cum = outp.tile([128, T], F32)
                nc.vector.tensor_tensor_scan(
                    out=cum,
                    data0=haz,
                    data1=zeros,
                    initial=base,
                    op0=mybir.AluOpType.add,
                    op1=mybir.AluOpType.add,
                )
                nc.scalar.dma_start(out=out_d[128 * b : 128 * (b + 1), :], in_=cum)

    nc.compile()
    return nc


_NC_CACHE = None


def kernel(x, W_hazard, b_hazard, W_base, b_base):
    global _NC_CACHE
    if _NC_CACHE is None:
        _NC_CACHE = _build_program()
    nc = _NC_CACHE

    x = np.asarray(x, dtype=np.float32)
    W_cat = np.concatenate(
        [np.asarray(W_hazard, np.float32), np.asarray(W_base, np.float32)], axis=0
    )  # [513, 2048]
    bias_row = np.concatenate(
        [np.asarray(b_hazard, np.float32), np.asarray(b_base, np.float32)]
    )  # [513]
    wt = np.concatenate([W_cat.T, bias_row[None, :]], axis=0)  # [2049, 513]
    wt = np.ascontiguousarray(
        np.concatenate([wt, np.zeros((D + 1, TP - (T + 1)), np.float32)], axis=1)
    )  # [2049, 516]

    in_maps = [
        {
            "xt": np.ascontiguousarray(x[BLOC * i : BLOC * (i + 1)].T),
            "wt": wt,
        }
        for i in range(NCORES)
    ]
    res = run_bass_kernel_spmd(nc, in_maps, list(range(NCORES)))
    return np.concatenate([res.results[i]["out"] for i in range(NCORES)], axis=0)


# revision 29
# speedup vs baseline: 1.1077x; 1.0401x over previous
"""Cumulative-probability head on 8 Trainium2 NeuronCores.

out[b, j] = sum_{i<=j} relu(x @ W_h^T + b_h)[b, i] + (x @ W_base^T + b_base)[b]

Data-parallel: x is sharded along batch (1024 rows per core); the small
weights are replicated. The host passes x pre-transposed per shard
([D, 1024], contiguous rows) so the contraction dim lands on SBUF
partitions with no on-device transposes. Per core:

  1. haz = xT.T @ WT_aug in float32r (FP22 multiplies, fp32 PSUM accum).
     WT_aug is [2049, 516]: hazard cols 0..511, base col 512, 3 zero pad
     cols; row 2048 is the bias row, added post-accumulation on DVE
     via a partition-broadcast read. The 516 output cols split into two
     even N=258 PSUM banks (fp32r requires an even moving dim).
  2. Each 128-row batch tile accumulates the full K=2048 contraction
     in a pair of PSUM banks; four tiles are in flight (8 banks), so
     the PE has work while input chunks stream in.
  3. Input DMAs are spread over three queue rings (Sync HWDGE, Scalar
     HWDGE, GPSIMD SWDGE) in k-order — one ring caps at ~160 GB/s,
     which would serialize the 12 MB of per-core input.
  4. ReLU on ScalarE (hazard cols only; base col stays unactivated),
     then the inclusive cumsum along T on DVE via tensor_tensor_scan
     with the base hazard as the per-partition initial state.
"""

import numpy as np

import concourse.bass as bass
import concourse.tile as tile
from concourse import bacc, mybir
from concourse.bass_utils import run_bass_kernel_spmd

B, D, T = 8192, 2048, 512
NCORES = 8
BLOC = B // NCORES            # 1024 rows per core
NB = BLOC // 128              # 8 batch tiles per core
NK = D // 128                 # 16 contraction chunks
TP = 516                      # padded output width (512 hazard + base + 3 junk)
NA = 258                      # output cols 0..257 in PSUM bank A
NBK = TP - NA                 # cols 258..515 in bank B (col 512 = base)
BOFF = T - NA                 # offset of the base col inside bank B (254)

F32 = mybir.dt.float32
F32R = mybir.dt.float32r


def _build_program():
    nc = bacc.Bacc("TRN2", target_bir_lowering=False, debug=False)

    xt_d = nc.dram_tensor("xt", [D, BLOC], F32R, kind="ExternalInput")
    wt_d = nc.dram_tensor("wt", [D + 1, TP], F32R, kind="ExternalInput")
    out_d = nc.dram_tensor("out", [BLOC, T], F32, kind="ExternalOutput")

    with tile.TileContext(nc) as tc:
        with (
            tc.tile_pool(name="consts", bufs=1) as consts,
            tc.tile_pool(name="wt", bufs=1) as wtp,
            tc.tile_pool(name="xt", bufs=1) as xtp,
            tc.tile_pool(name="haz", bufs=4) as hazp,
            tc.tile_pool(name="outp", bufs=4) as outp,
            tc.tile_pool(name="ps_mm", bufs=4, space="PSUM") as ps_mm,
        ):
            zeros = consts.tile([128, T], F32)
            nc.vector.memset(zeros, 0.0)

            # Input loads, k-ordered, spread over three DMA rings. The two
            # HWDGE rings (Sync, Scalar) are faster than the GPSIMD SWDGE
            # ring, so the first chunks — which gate the PE pipeline ramp —
            # go to the HWDGE rings, and the slow ring only carries late
            # chunks. Weights (half the size of an x chunk) ride opposite
            # rings from their x chunk so the pair lands together.
            XT_RING = [0, 1, 0, 1, 2, 0, 1, 2, 0, 1, 2, 0, 1, 2, 0, 1]
            WT_RING = [1, 0, 1, 0, 0, 1, 2, 0, 1, 2, 0, 1, 2, 0, 1, 2]
            rings = [nc.sync, nc.scalar, nc.gpsimd]
            xt_tiles = []
            wt_tiles = []
            wbias_bc = wtp.tile([128, TP], F32, tag="wbias")
            for k in range(NK):
                xk = xtp.tile([128, BLOC], F32R, tag=f"xt{k}")
                rings[XT_RING[k]].dma_start(out=xk, in_=xt_d[128 * k : 128 * (k + 1), :])
                xt_tiles.append(xk)
                w = wtp.tile([128, TP], F32R, tag=f"wt{k}")
                rings[WT_RING[k]].dma_start(out=w, in_=wt_d[128 * k : 128 * (k + 1), :])
                wt_tiles.append(w)
                if k == 8:
                    # Bias row replicated across all 128 partitions with a
                    # partition-stride-0 DMA read (engines can't read
                    # stride-0 partition APs, but DMA can). Emitted mid-
                    # stream on the GPSIMD ring: early enough to be resident
                    # long before the first b-tile's bias add (which gates
                    # PSUM bank recycling), late enough not to delay the
                    # chunks that pace the PE ramp.
                    bias_src = wt_d[D : D + 1, :]
                    nc.gpsimd.dma_start(
                        out=wbias_bc,
                        in_=bass.AP(
                            tensor=bias_src.tensor,
                            offset=bias_src.offset,
                            ap=[[0, 128]] + list(bias_src.ap[1:]),
                        ).bitcast(F32),
                    )

            # Full-K accumulation per 128-row batch tile; bufs=4 on each
            # PSUM tag -> 4 b-tiles in flight across all 8 banks.
            for b in range(NB):
                pA = ps_mm.tile([128, NA], F32, tag="pA")
                pB = ps_mm.tile([128, NBK], F32, tag="pB")
                for k in range(NK):
                    xt_ap = xt_tiles[k][:, 128 * b : 128 * (b + 1)]
                    w = wt_tiles[k]
                    nc.tensor.matmul(
                        pA[:], xt_ap, w[:, 0:NA],
                        start=(k == 0), stop=(k == NK - 1),
                    )
                    nc.tensor.matmul(
                        pB[:], xt_ap, w[:, NA:TP],
                        start=(k == 0), stop=(k == NK - 1),
                    )

                # Bias row added on DVE via a partition-broadcast read —
                # keeps the K=1 ones-row matmuls (and their LDWEIGHTS)
                # off the PE stream.
                pre = hazp.tile([128, 2, NA], F32, tag="pre")
                nc.vector.tensor_add(pre[:, 0, :], pA[:], wbias_bc[:, 0:NA])
                nc.vector.tensor_add(pre[:, 1, :], pB[:], wbias_bc[:, NA:TP])

                haz = hazp.tile([128, T], F32, tag="haz")
                base = hazp.tile([128, 1], F32, tag="base")
                nc.scalar.activation(
                    out=haz[:, 0:NA], in_=pre[:, 0, :],
                    func=mybir.ActivationFunctionType.Relu,
                )
                nc.scalar.activation(
                    out=haz[:, NA:T], in_=pre[:, 1, 0:BOFF],
                    func=mybir.ActivationFunctionType.Relu,
                )
                nc.scalar.copy(out=base, in_=pre[:, 1, BOFF : BOFF + 1])

                cum = outp.tile([128, T], F32)
                nc.vector.tensor_tensor_scan(
                    out=cum,
                    data0=haz,
                    data1=zeros,
                    initial=base,
                    op0=mybir.AluOpType.add,
                    op1=mybir.AluOpType.add,
                )
                nc.scalar.dma_start(out=out_d[128 * b : 128 * (b + 1), :], in_=cum)

    nc.compile()
    return nc


_NC_CACHE = None


def kernel(x, W_hazard, b_hazard, W_base, b_base):
    global _NC_CACHE
    if _NC_CACHE is None:
        _NC_CACHE = _build_program()
    nc = _NC_CACHE

    x = np.asarray(x, dtype=np.float32)
    W_cat = np.concatenate(
        [np.asarray(W_hazard, np.float32), np.asarray(W_base, np.float32)], axis=0
    )  # [513, 2048]
    bias_row = np.concatenate(
        [np.asarray(b_hazard, np.float32), np.asarray(b_base, np.float32)]
    )  # [513]
    wt = np.concatenate([W_cat.T, bias_row[None, :]], axis=0)  # [2049, 513]
    wt = np.ascontiguousarray(
        np.concatenate([wt, np.zeros((D + 1, TP - (T + 1)), np.float32)], axis=1)
    )  # [2049, 516]

    in_maps = [
        {
            "xt": np.ascontiguousarray(x[BLOC * i : BLOC * (i + 1)].T),
            "wt": wt,
        }
        for i in range(NCORES)
    ]
    res = run_bass_kernel_spmd(nc, in_maps, list(range(NCORES)))
    return np.concatenate([res.results[i]["out"] for i in range(NCORES)], axis=0)


# revision 30
# speedup vs baseline: 1.1723x; 1.0582x over previous
"""Cumulative-probability head on 8 Trainium2 NeuronCores.

out[b, j] = sum_{i<=j} relu(x @ W_h^T + b_h)[b, i] + (x @ W_base^T + b_base)[b]

Data-parallel: x is sharded along batch (1024 rows per core); the small
weights are replicated. The host passes x pre-transposed per shard
([D, 1024], contiguous rows) so the contraction dim lands on SBUF
partitions with no on-device transposes. Per core:

  1. haz = xT.T @ WT_aug in float32r (FP22 multiplies, fp32 PSUM accum).
     WT_aug is [2049, 516]: hazard cols 0..511, base col 512, 3 zero pad
     cols; row 2048 is the bias row, added post-accumulation on DVE
     via a partition-broadcast read. The 516 output cols split into two
     even N=258 PSUM banks (fp32r requires an even moving dim).
  2. Each 128-row batch tile accumulates the full K=2048 contraction
     in a pair of PSUM banks; four tiles are in flight (8 banks), so
     the PE has work while input chunks stream in.
  3. Input DMAs are spread over three queue rings (Sync HWDGE, Scalar
     HWDGE, GPSIMD SWDGE) in k-order — one ring caps at ~160 GB/s,
     which would serialize the 12 MB of per-core input.
  4. ReLU on ScalarE (hazard cols only; base col stays unactivated),
     then the inclusive cumsum along T on DVE via tensor_tensor_scan
     with the base hazard as the per-partition initial state.
"""

import numpy as np

import concourse.bass as bass
import concourse.tile as tile
from concourse import bacc, mybir
from concourse.bass_utils import run_bass_kernel_spmd

B, D, T = 8192, 2048, 512
NCORES = 8
BLOC = B // NCORES            # 1024 rows per core
NB = BLOC // 128              # 8 batch tiles per core
NK = D // 128                 # 16 contraction chunks
TP = 516                      # padded output width (512 hazard + base + 3 junk)
NA = 258                      # output cols 0..257 in PSUM bank A
NBK = TP - NA                 # cols 258..515 in bank B (col 512 = base)
BOFF = T - NA                 # offset of the base col inside bank B (254)

F32 = mybir.dt.float32
F32R = mybir.dt.float32r


def _build_program():
    nc = bacc.Bacc("TRN2", target_bir_lowering=False, debug=False)

    xt_d = nc.dram_tensor("xt", [D, BLOC], F32R, kind="ExternalInput")
    wt_d = nc.dram_tensor("wt", [D + 1, TP], F32R, kind="ExternalInput")
    out_d = nc.dram_tensor("out", [BLOC, T], F32, kind="ExternalOutput")

    with tile.TileContext(nc) as tc:
        with (
            tc.tile_pool(name="consts", bufs=1) as consts,
            tc.tile_pool(name="wt", bufs=1) as wtp,
            tc.tile_pool(name="xt", bufs=1) as xtp,
            tc.tile_pool(name="haz", bufs=4) as hazp,
            tc.tile_pool(name="outp", bufs=4) as outp,
            tc.tile_pool(name="ps_mm", bufs=4, space="PSUM") as ps_mm,
        ):
            zeros = consts.tile([128, T], F32)
            nc.vector.memset(zeros, 0.0)

            # Input loads, k-ordered, spread over three DMA rings. The two
            # HWDGE rings (Sync, Scalar) are faster than the GPSIMD SWDGE
            # ring, so the first chunks — which gate the PE pipeline ramp —
            # go to the HWDGE rings, and the slow ring only carries late
            # chunks. Weights (half the size of an x chunk) ride opposite
            # rings from their x chunk so the pair lands together.
            XT_RING = [0, 1, 0, 1, 2, 0, 1, 2, 0, 1, 2, 0, 1, 2, 0, 1]
            WT_RING = [1, 0, 1, 0, 0, 1, 2, 0, 1, 2, 0, 1, 2, 0, 1, 2]
            rings = [nc.sync, nc.scalar, nc.gpsimd]
            xt_tiles = []
            wt_tiles = []
            wbias_bc = wtp.tile([128, TP], F32, tag="wbias")
            H = BLOC // 2
            for k in range(NK):
                # x chunks load in column halves: the first four (live)
                # batch tiles only read cols 0..511, so streaming ALL
                # first-halves before any second-half lets them retire on
                # half the input volume and frees PSUM banks mid-window
                # for batch tiles 4..7.
                xk = xtp.tile([128, BLOC], F32R, tag=f"xt{k}")
                rings[XT_RING[k]].dma_start(
                    out=xk[:, 0:H], in_=xt_d[128 * k : 128 * (k + 1), 0:H]
                )
                xt_tiles.append(xk)
                w = wtp.tile([128, TP], F32R, tag=f"wt{k}")
                rings[WT_RING[k]].dma_start(out=w, in_=wt_d[128 * k : 128 * (k + 1), :])
                wt_tiles.append(w)
                if k == 8:
                    # Bias row replicated across all 128 partitions with a
                    # partition-stride-0 DMA read (engines can't read
                    # stride-0 partition APs, but DMA can). Emitted mid-
                    # stream on the GPSIMD ring: early enough to be resident
                    # long before the first b-tile's bias add (which gates
                    # PSUM bank recycling), late enough not to delay the
                    # chunks that pace the PE ramp.
                    bias_src = wt_d[D : D + 1, :]
                    nc.gpsimd.dma_start(
                        out=wbias_bc,
                        in_=bass.AP(
                            tensor=bias_src.tensor,
                            offset=bias_src.offset,
                            ap=[[0, 128]] + list(bias_src.ap[1:]),
                        ).bitcast(F32),
                    )
            for k in range(NK):
                rings[XT_RING[k]].dma_start(
                    out=xt_tiles[k][:, H:BLOC],
                    in_=xt_d[128 * k : 128 * (k + 1), H:BLOC],
                )

            # Full-K accumulation per 128-row batch tile; bufs=4 on each
            # PSUM tag -> 4 b-tiles in flight across all 8 banks.
            for b in range(NB):
                pA = ps_mm.tile([128, NA], F32, tag="pA")
                pB = ps_mm.tile([128, NBK], F32, tag="pB")
                for k in range(NK):
                    xt_ap = xt_tiles[k][:, 128 * b : 128 * (b + 1)]
                    w = wt_tiles[k]
                    nc.tensor.matmul(
                        pA[:], xt_ap, w[:, 0:NA],
                        start=(k == 0), stop=(k == NK - 1),
                    )
                    nc.tensor.matmul(
                        pB[:], xt_ap, w[:, NA:TP],
                        start=(k == 0), stop=(k == NK - 1),
                    )

                # Bias row added on DVE via a partition-broadcast read —
                # keeps the K=1 ones-row matmuls (and their LDWEIGHTS)
                # off the PE stream.
                pre = hazp.tile([128, 2, NA], F32, tag="pre")
                nc.vector.tensor_add(pre[:, 0, :], pA[:], wbias_bc[:, 0:NA])
                nc.vector.tensor_add(pre[:, 1, :], pB[:], wbias_bc[:, NA:TP])

                haz = hazp.tile([128, T], F32, tag="haz")
                base = hazp.tile([128, 1], F32, tag="base")
                nc.scalar.activation(
                    out=haz[:, 0:NA], in_=pre[:, 0, :],
                    func=mybir.ActivationFunctionType.Relu,
                )
                nc.scalar.activation(
                    out=haz[:, NA:T], in_=pre[:, 1, 0:BOFF],
                    func=mybir.ActivationFunctionType.Relu,
                )
                nc.scalar.copy(out=base, in_=pre[:, 1, BOFF : BOFF + 1])

                cum = outp.tile([128, T], F32)
                nc.vector.tensor_tensor_scan(
                    out=cum,
                    data0=haz,
                    data1=zeros,
                    initial=base,
                    op0=mybir.AluOpType.add,
                    op1=mybir.AluOpType.add,
                )
                nc.scalar.dma_start(out=out_d[128 * b : 128 * (b + 1), :], in_=cum)

    nc.compile()
    return nc


_NC_CACHE = None


def kernel(x, W_hazard, b_hazard, W_base, b_base):
    global _NC_CACHE
    if _NC_CACHE is None:
        _NC_CACHE = _build_program()
    nc = _NC_CACHE

    x = np.asarray(x, dtype=np.float32)
    W_cat = np.concatenate(
        [np.asarray(W_hazard, np.float32), np.asarray(W_base, np.float32)], axis=0
    )  # [513, 2048]
    bias_row = np.concatenate(
        [np.asarray(b_hazard, np.float32), np.asarray(b_base, np.float32)]
    )  # [513]
    wt = np.concatenate([W_cat.T, bias_row[None, :]], axis=0)  # [2049, 513]
    wt = np.ascontiguousarray(
        np.concatenate([wt, np.zeros((D + 1, TP - (T + 1)), np.float32)], axis=1)
    )  # [2049, 516]

    in_maps = [
        {
            "xt": np.ascontiguousarray(x[BLOC * i : BLOC * (i + 1)].T),
            "wt": wt,
        }
        for i in range(NCORES)
    ]
    res = run_bass_kernel_spmd(nc, in_maps, list(range(NCORES)))
    return np.concatenate([res.results[i]["out"] for i in range(NCORES)], axis=0)
